# revision 1
# baseline (speedup 1.0000x reference)
"""Trainium2 Bass kernel for the EnhancedMamba2Mixer problem.

Sharding: 2-way data parallel over batch x 4-way tensor parallel over heads
(16 heads / 1024 INTER channels per core).  All 8 cores run one identical
Bass program on different input slices; no collectives.  The RMSNorm rsqrt
is a per-row scalar that commutes with the linear out_proj, so each core
emits a partial out_proj product plus its local gated activations z; the
host sums partials, computes the variance from z, and applies the scale.

Device program per core (chunked SSD scan, chunk L=128):
  in_proj (gate|hs columns, bf16) -> causal depthwise conv + SiLU
  -> per chunk: G^T = B @ C^T (shared across heads since n_groups=1),
     per head: M^T = G^T * E^T (host-precomputed decay mask),
     Y^T = X~^T M + (S_prev C^T) * exp(cum), state' = dA*state + B^T X2
  -> z = y * silu(gate) -> partial out_proj.
The sequence is processed in two halves; emission interleaves the second
half's in_proj (and the first half's out_proj) between scan chunks so the
PE stays busy while DVE/GPSIMD grind the scan elementwise work.
Small exp/softplus/cumsum tensors (0.1% of FLOPs) are prepared on host in
fp64 to protect the exponential-path precision.
"""
import sys

sys.path.insert(0, "/opt/trn_rl_repo")

from contextlib import ExitStack

import ml_dtypes
import numpy as np

import concourse.bass as bass  # noqa: F401
import concourse.mybir as mybir
import concourse.tile as tile
from concourse import bacc
from concourse.masks import make_identity

HID = 2048
INTER = 4096
NH = 64
HD = 64
NST = 128
KCV = 4
EPS = 1e-5
B = 2
S = 2048
L = 128
NCHUNK = S // L
NCORES = 8
HLOC = 16
CLOC = 1024
HALF = S // 2

BF16 = mybir.dt.bfloat16
F32 = mybir.dt.float32
bfnp = ml_dtypes.bfloat16
MUL = mybir.AluOpType.mult
ADD = mybir.AluOpType.add
SIGM = mybir.ActivationFunctionType.Sigmoid

_CACHE = {}


def _build_program():
    nc = bacc.Bacc("TRN2", target_bir_lowering=False, debug=False,
                   num_devices=NCORES)

    def din(name, shape, dt):
        return nc.dram_tensor(name, shape, dt, kind="ExternalInput").ap()

    # all layouts are pre-arranged on host so each DMA is contiguous
    # within a partition row
    XT = din("xt", [2, 128, 16, HALF], BF16)        # [hf][p][k][t]
    WT = din("wt", [16, 128, 16, 128], BF16)        # [j][p][k][c]
    WO = din("wo", [16, 128, 8, 128], BF16)         # [o][p][kj][c]
    BT = din("bt", [NST, S], BF16)                  # B^T post conv+silu
    CT = din("ct", [NST, S], BF16)                  # C^T post conv+silu
    BR = din("br", [S, NST], BF16)                  # B rows
    ETD = din("et", [NCHUNK, L, HLOC, L], BF16)     # [cg][s][h][t] decay mask
    EXC = din("exc", [NCHUNK, L, 8, L], BF16)       # [cg][drow][p][t] exp(cum)
    DTP = din("dtp", [NCHUNK, 2, L, 8, L], BF16)    # [cg][v][s][p][dcol]
    DAC = din("dac", [128, NCHUNK * HLOC], F32)     # exp(cumL) replicated
    DCO = din("dco", [128, 8], F32)                 # D per pair, row-split
    CW = din("cw", [8, 128, KCV], F32)
    CB = din("cb", [8, 128], F32)
    OUTT = nc.dram_tensor("outt", [HID, S], F32, kind="ExternalOutput").ap()
    ZT = nc.dram_tensor("ztout", [2, 128, 8, HALF], BF16,
                        kind="ExternalOutput").ap()

    with tile.TileContext(nc) as tc, ExitStack() as ctx:
        P = ctx.enter_context
        const = P(tc.tile_pool(name="const", bufs=1))
        wpool = P(tc.tile_pool(name="wpool", bufs=3))
        wopool = P(tc.tile_pool(name="wopool", bufs=3))
        xpool = P(tc.tile_pool(name="xpool", bufs=1))
        hpool = P(tc.tile_pool(name="hpool", bufs=1))
        sgpool = P(tc.tile_pool(name="sgpool", bufs=2))
        etpool = P(tc.tile_pool(name="etpool", bufs=2))
        brpool = P(tc.tile_pool(name="brpool", bufs=2))
        scr = P(tc.tile_pool(name="scr", bufs=3))
        cscr = P(tc.tile_pool(name="cscr", bufs=2))
        oev = P(tc.tile_pool(name="oev", bufs=2))
        mm_ps = P(tc.tile_pool(name="mm_ps", bufs=3, space="PSUM"))
        xt_ps = P(tc.tile_pool(name="xt_ps", bufs=2, space="PSUM"))
        y_ps = P(tc.tile_pool(name="y_ps", bufs=1, space="PSUM"))
        z_ps = P(tc.tile_pool(name="z_ps", bufs=1, space="PSUM"))
        st_ps = P(tc.tile_pool(name="st_ps", bufs=1, space="PSUM"))

        id128 = const.tile([128, 128], BF16)
        make_identity(nc, id128[:])
        dac_s = const.tile([128, NCHUNK * HLOC], F32)
        dco_s = const.tile([128, 8], F32)
        cw_s = const.tile([128, 8, KCV], F32)
        cb_s = const.tile([128, 8], F32)
        bt_s = const.tile([128, S], BF16)
        ct_s = const.tile([128, S], BF16)
        state = const.tile([128, HLOC, HD], BF16)
        nc.gpsimd.memset(state[:], 0.0)
        carry = const.tile([128, 8, 3], BF16)
        nc.gpsimd.memset(carry[:], 0.0)

        def load_consts():
            nc.sync.dma_start(dac_s[:], DAC)
            nc.sync.dma_start(dco_s[:], DCO)
            nc.sync.dma_start(cw_s[:], CW.rearrange("j p k -> p j k"))
            nc.sync.dma_start(cb_s[:], CB.rearrange("j p -> p j"))
            nc.sync.dma_start(bt_s[:], BT)
            nc.sync.dma_start(ct_s[:], CT)

        halfbuf = {}

        def alloc_half(hf):
            xh = xpool.tile([128, 16, HALF], BF16, tag="xh")
            for k in range(16):
                nc.sync.dma_start(xh[:, k, :], XT[hf][:, k, :])
            sg = sgpool.tile([128, 8, HALF], BF16, tag="sg")
            hraw = hpool.tile([128, 8, HALF + 3], BF16, tag="hraw")
            nc.vector.tensor_copy(hraw[:, :, 0:3], carry[:])
            halfbuf[hf] = dict(xh=xh, sg=sg, hraw=hraw)

        def inproj_j(hf, j):
            hb = halfbuf[hf]
            wst = wpool.tile([128, 16, 128], BF16, tag="wst")
            nc.sync.dma_start(wst[:], WT[j])
            for s2 in range(2):
                ps = mm_ps.tile([128, 512], F32, tag="mmps")
                for k in range(16):
                    nc.tensor.matmul(
                        ps[:], wst[:, k, :],
                        hb["xh"][:, k, s2 * 512:(s2 + 1) * 512],
                        start=(k == 0), stop=(k == 15))
                dst = slice(s2 * 512, (s2 + 1) * 512)
                if j < 8:
                    sig = cscr.tile([128, 512], BF16, tag="sig")
                    nc.scalar.activation(sig[:], ps[:], SIGM)
                    nc.vector.tensor_mul(hb["sg"][:, j, dst], ps[:], sig[:])
                else:
                    nc.scalar.copy(hb["hraw"][:, j - 8, 3 + s2 * 512:
                                              3 + (s2 + 1) * 512], ps[:])

        def conv_half(hf):
            hb = halfbuf[hf]
            hraw = hb["hraw"]
            if hf == 0:
                nc.vector.tensor_copy(carry[:], hraw[:, :, HALF:HALF + 3])
            hconv = hpool.tile([128, 8, HALF], BF16, tag="hconv")
            for hj in range(8):
                a1 = cscr.tile([128, HALF], F32, tag="cacc1")
                nc.vector.tensor_scalar(a1[:], hraw[:, hj, 0:HALF],
                                        cw_s[:, hj, 0:1], cb_s[:, hj:hj + 1],
                                        MUL, ADD)
                a2 = cscr.tile([128, HALF], F32, tag="cacc2")
                nc.vector.scalar_tensor_tensor(a2[:], hraw[:, hj, 1:HALF + 1],
                                               cw_s[:, hj, 1:2], a1[:],
                                               MUL, ADD)
                a3 = cscr.tile([128, HALF], F32, tag="cacc1")
                nc.vector.scalar_tensor_tensor(a3[:], hraw[:, hj, 2:HALF + 2],
                                               cw_s[:, hj, 2:3], a2[:],
                                               MUL, ADD)
                a4 = cscr.tile([128, HALF], F32, tag="cacc2")
                nc.vector.scalar_tensor_tensor(a4[:], hraw[:, hj, 3:HALF + 3],
                                               cw_s[:, hj, 3:4], a3[:],
                                               MUL, ADD)
                sig4 = cscr.tile([128, HALF], BF16, tag="csig")
                nc.scalar.activation(sig4[:], a4[:], SIGM)
                nc.vector.tensor_mul(hconv[:, hj, :], a4[:], sig4[:])
            hb["hconv"] = hconv

        def scan_chunk(hf, cl):
            hb = halfbuf[hf]
            hconv = hb["hconv"]
            sg = hb["sg"]
            cg = hf * 8 + cl
            t0 = cg * 128
            tl_ = slice(cl * 128, (cl + 1) * 128)
            et = etpool.tile([128, HLOC, 128], BF16, tag="et")
            nc.sync.dma_start(et[:], ETD[cg])
            exc = etpool.tile([128, 8, 128], BF16, tag="exc")
            nc.sync.dma_start(exc[:], EXC[cg])
            dtp = etpool.tile([128, 2, 8, 128], BF16, tag="dtp")
            nc.sync.dma_start(dtp[:], DTP[cg].rearrange("v s p c -> s v p c"))
            br = brpool.tile([128, 128], BF16, tag="br")
            nc.sync.dma_start(br[:], BR[t0:t0 + 128, :])
            gps = xt_ps.tile([128, 128], F32, tag="xtps")
            nc.tensor.matmul(gps[:], bt_s[:, t0:t0 + 128],
                             ct_s[:, t0:t0 + 128], start=True, stop=True)
            gs = scr.tile([128, 128], F32, tag="gs")
            nc.scalar.copy(gs[:], gps[:])
            for p in range(8):
                heads = (2 * p, 2 * p + 1)
                xtp = xt_ps.tile([128, 128], BF16, tag="xtps")
                nc.tensor.transpose(xtp[:], hconv[:, p, tl_], id128[:])
                xx = scr.tile([128, 2, 128], BF16, tag="xx")
                nc.vector.tensor_mul(
                    xx[:], xtp[:].unsqueeze(1).broadcast_to([128, 2, 128]),
                    dtp[:, :, p, :])
                xpair, x2pair = xx[:, 0, :], xx[:, 1, :]
                mtp = scr.tile([128, 2, 128], BF16, tag="mtp")
                nc.gpsimd.tensor_mul(
                    mtp[:], gs[:].unsqueeze(1).broadcast_to([128, 2, 128]),
                    et[:, 2 * p:2 * p + 2, :])
                mts = [mtp[:, 0, :], mtp[:, 1, :]]
                yps = y_ps.tile([128, 128], F32, tag="yps")
                zps = z_ps.tile([128, 128], F32, tag="zps")
                stp = st_ps.tile([128, 128], F32, tag="stps")
                for hh, hl in enumerate(heads):
                    dsl = slice(hh * 64, (hh + 1) * 64)
                    nc.tensor.matmul(yps[dsl, :], xpair[:, dsl], mts[hh],
                                     start=True, stop=True)
                    nc.tensor.matmul(zps[dsl, :], state[:, hl, :],
                                     ct_s[:, t0:t0 + 128],
                                     start=True, stop=True)
                    nc.tensor.matmul(stp[:, dsl], br[:], x2pair[:, dsl],
                                     start=True, stop=True)
                for hh, hl in enumerate(heads):
                    dsl = slice(hh * 64, (hh + 1) * 64)
                    idx = cg * HLOC + hl
                    nc.vector.scalar_tensor_tensor(
                        state[:, hl, :], state[:, hl, :],
                        dac_s[:, idx:idx + 1], stp[:, dsl], MUL, ADD)
                t1 = scr.tile([128, 128], F32, tag="t1")
                nc.vector.tensor_mul(t1[:], zps[:], exc[:, p, :])
                t2 = scr.tile([128, 128], F32, tag="t2")
                nc.vector.scalar_tensor_tensor(
                    t2[:], hconv[:, p, tl_], dco_s[:, p:p + 1], yps[:],
                    MUL, ADD)
                yv = scr.tile([128, 128], F32, tag="yv")
                nc.gpsimd.tensor_add(yv[:], t1[:], t2[:])
                nc.gpsimd.tensor_mul(sg[:, p, tl_], yv[:], sg[:, p, tl_])

        def outproj_unit(hf, o, s2):
            zb = halfbuf[hf]["sg"]
            wo = wopool.tile([128, 8, 128], BF16, tag="wo")
            nc.sync.dma_start(wo[:], WO[o])
            q0 = hf * HALF
            ps = mm_ps.tile([128, 512], F32, tag="mmps")
            for kj in range(8):
                nc.tensor.matmul(ps[:], wo[:, kj, :],
                                 zb[:, kj, s2 * 512:(s2 + 1) * 512],
                                 start=(kj == 0), stop=(kj == 7))
            ov = oev.tile([128, 512], F32, tag="oev")
            nc.scalar.copy(ov[:], ps[:])
            nc.sync.dma_start(
                OUTT[o * 128:(o + 1) * 128,
                     q0 + s2 * 512:q0 + (s2 + 1) * 512], ov[:])

        # ---- phase 0: in_proj + conv of half 0 ----
        alloc_half(0)
        for j in range(3):
            inproj_j(0, j)
        load_consts()
        for j in range(3, 16):
            inproj_j(0, j)
        conv_half(0)

        # ---- phase 1: scan half 0, interleaved with in_proj half 1 ----
        alloc_half(1)
        for cl in range(8):
            scan_chunk(0, cl)
            inproj_j(1, 2 * cl)
            inproj_j(1, 2 * cl + 1)
        # ---- phase 2: scan half 1, interleaved with out_proj half 0;
        # late chunks also pull in half-1 s2=0 units (they only need the
        # first 4 chunks of sg1) ----
        conv_half(1)
        for cl in range(8):
            scan_chunk(1, cl)
            outproj_unit(0, 2 * cl, 0)
            outproj_unit(0, 2 * cl, 1)
            outproj_unit(0, 2 * cl + 1, 0)
            outproj_unit(0, 2 * cl + 1, 1)
            if cl >= 4:
                o2 = (cl - 4) * 4
                for oo in range(o2, o2 + 4):
                    outproj_unit(1, oo, 0)
        # ---- phase 3: out_proj half 1, s2=1 ----
        for o in range(16):
            outproj_unit(1, o, 1)
        nc.sync.dma_start(ZT[0], halfbuf[0]["sg"][:])
        nc.sync.dma_start(ZT[1], halfbuf[1]["sg"][:])

    nc.compile()
    return nc


def _softplus64(x):
    x = np.asarray(x, np.float64)
    return np.where(x > 30, x, np.log1p(np.exp(np.minimum(x, 30.0))))


def _silu(x):
    return x / (1.0 + np.exp(-x))


def _causal_conv(x, w, b):
    pad = np.zeros((KCV - 1, x.shape[1]), x.dtype)
    xp = np.concatenate([pad, x], axis=0)
    out = np.zeros_like(x)
    for k in range(KCV):
        out += xp[k:k + x.shape[0]] * w[None, :, k]
    return out + b[None, :]


def _host_prep(inputs):
    hs = np.asarray(inputs["hidden_states"], np.float32)
    W = np.asarray(inputs["in_proj_w"], np.float32)
    cw = np.asarray(inputs["conv_w"], np.float32)[:, 0, :]
    cb = np.asarray(inputs["conv_b"], np.float32)
    dt_bias = np.asarray(inputs["dt_bias"], np.float64)
    A = -np.exp(np.asarray(inputs["A_log"], np.float64))
    D = np.asarray(inputs["D"], np.float32)
    nw = np.asarray(inputs["norm_weight"], np.float32)
    Wout = np.asarray(inputs["out_proj_w"], np.float32)

    Wg = W[0:INTER]
    Whs = W[INTER:2 * INTER]
    Wbc = W[2 * INTER:2 * INTER + 2 * NST]
    Wdt = W[2 * INTER + 2 * NST:]

    per_batch = []
    for b in range(B):
        x = hs[b]
        bc_raw = x @ Wbc.T
        dt_raw = x @ Wdt.T
        bc = _silu(_causal_conv(bc_raw, cw[INTER:], cb[INTER:]))
        Bm = bc[:, :NST].astype(np.float32)
        Cm = bc[:, NST:].astype(np.float32)
        dt = _softplus64(dt_raw.astype(np.float64) + dt_bias[None, :])
        cum = (dt * A[None, :]).reshape(NCHUNK, L, NH).cumsum(axis=1)
        per_batch.append((x, Bm, Cm, dt, cum))

    sidx = np.arange(L)
    causal = (sidx[None, :] >= sidx[:, None])     # keep t >= s
    in_maps = []
    for core in range(NCORES):
        b, tp = divmod(core, 4)
        x, Bm, Cm, dt, cum = per_batch[b]
        csel = slice(tp * CLOC, (tp + 1) * CLOC)
        h0 = tp * HLOC
        cuml = cum[:, :, h0:h0 + HLOC]            # [c, i, 16] f64
        dtl = dt[:, h0:h0 + HLOC].reshape(NCHUNK, L, HLOC)

        # E^T mask, laid out [cg][s][h][t]
        diff = cuml[:, None, :, :] - cuml[:, :, None, :]   # [c, s, t, h]
        et = np.exp(np.where(causal[None, :, :, None], diff, -np.inf))
        et = np.transpose(et, (0, 1, 3, 2)).astype(bfnp)   # [c, s, h, t]

        expc = np.exp(cuml)                                # [c, t, h]
        exc = np.empty((NCHUNK, L, 8, L), np.float32)      # [c, drow, p, t]
        for p in range(8):
            exc[:, 0:64, p, :] = expc[:, None, :, 2 * p]
            exc[:, 64:128, p, :] = expc[:, None, :, 2 * p + 1]
        exc = exc.astype(bfnp)

        dtp = np.empty((NCHUNK, 2, L, 8, L), np.float32)
        x2s = (dtl * np.exp(cuml[:, -1:, :] - cuml)).astype(np.float32)
        dtf = dtl.astype(np.float32)
        for p in range(8):
            dtp[:, 0, :, p, 0:64] = dtf[:, :, 2 * p, None]
            dtp[:, 0, :, p, 64:128] = dtf[:, :, 2 * p + 1, None]
            dtp[:, 1, :, p, 0:64] = x2s[:, :, 2 * p, None]
            dtp[:, 1, :, p, 64:128] = x2s[:, :, 2 * p + 1, None]

        dac = np.broadcast_to(
            np.exp(cuml[:, -1, :]).reshape(1, NCHUNK * HLOC),
            (128, NCHUNK * HLOC)).astype(np.float32).copy()

        dco = np.empty((128, 8), np.float32)
        for p in range(8):
            dco[0:64, p] = D[h0 + 2 * p]
            dco[64:128, p] = D[h0 + 2 * p + 1]

        wt = np.concatenate([Wg[csel], Whs[csel]], axis=0)     # [2048c, hid]
        # [j][p][k][c]: W^T chunked -> wt[c_col, hid] with c_col=j*128+c,
        # hid=k*128+p
        wt4 = np.transpose(wt.reshape(16, 128, 16, 128), (0, 3, 2, 1))
        wo = (Wout[:, csel] * nw[None, csel])                  # [o, j]
        # [o_t][p][kj][c]: lhsT[j, o] with j=kj*128+p, o=o_t*128+c
        wo4 = np.transpose(wo.reshape(16, 128, 8, 128), (0, 3, 2, 1))
        # xt [hf][p][k][t]: x^T[hid, t] with hid=k*128+p
        xt4 = np.transpose(
            np.ascontiguousarray(x.T).reshape(16, 128, 2, HALF), (2, 1, 0, 3))

        in_maps.append({
            "xt": np.ascontiguousarray(xt4).astype(bfnp),
            "wt": np.ascontiguousarray(wt4).astype(bfnp),
            "wo": np.ascontiguousarray(wo4).astype(bfnp),
            "bt": np.ascontiguousarray(Bm.T).astype(bfnp),
            "ct": np.ascontiguousarray(Cm.T).astype(bfnp),
            "br": Bm.astype(bfnp),
            "et": np.ascontiguousarray(et),
            "exc": np.ascontiguousarray(exc),
            "dtp": dtp.astype(bfnp),
            "dac": dac,
            "dco": dco,
            "cw": np.ascontiguousarray(cw[csel].reshape(8, 128, KCV)),
            "cb": np.ascontiguousarray(cb[csel].reshape(8, 128)),
        })
    return in_maps


def _get_runner(nc):
    """Cached jitted SPMD runner (mirrors bass2jax.run_bass_via_pjrt)."""
    if "runner" in _CACHE:
        return _CACHE["runner"]
    import jax
    from jax.sharding import Mesh, PartitionSpec
    from jax.experimental.shard_map import shard_map
    from concourse import bass2jax

    bass2jax.install_neuronx_cc_hook()
    partition_name = (nc.partition_id_tensor.name
                      if nc.partition_id_tensor else None)
    in_names, out_names, out_avals, zero_shapes = [], [], [], []
    for alloc in nc.m.functions[0].allocations:
        if not isinstance(alloc, mybir.MemoryLocationSet):
            continue
        name = alloc.memorylocations[0].name
        if alloc.kind == "ExternalInput":
            if name != partition_name:
                in_names.append(name)
        elif alloc.kind == "ExternalOutput":
            out_names.append(name)
            shape = tuple(alloc.tensor_shape)
            dtype = mybir.dt.np(alloc.dtype)
            out_avals.append(jax.core.ShapedArray(shape, dtype))
            zero_shapes.append((shape, dtype))
    n_params = len(in_names)
    all_in_names = in_names + out_names
    if partition_name is not None:
        all_in_names = all_in_names + [partition_name]
    donate = tuple(range(n_params, n_params + len(out_names)))

    def _body(*args):
        operands = list(args)
        if partition_name is not None:
            operands.append(bass2jax.partition_id_tensor())
        outs = bass2jax._bass_exec_p.bind(
            *operands,
            out_avals=tuple(out_avals),
            in_names=tuple(all_in_names),
            out_names=tuple(out_names),
            lowering_input_output_aliases=(),
            sim_require_finite=True,
            sim_require_nnan=True,
            nc=nc,
        )
        return tuple(outs)

    devices = jax.devices()[:NCORES]
    mesh = Mesh(np.asarray(devices), ("core",))
    specs = (PartitionSpec("core"),) * (n_params + len(out_names))
    sharded = jax.jit(
        shard_map(_body, mesh=mesh, in_specs=specs,
                  out_specs=(PartitionSpec("core"),) * len(out_names),
                  check_rep=False),
        donate_argnums=donate, keep_unused=True)

    def run(in_maps):
        concat_in = [
            np.concatenate([np.asarray(m[name]) for m in in_maps], axis=0)
            for name in in_names
        ]
        concat_zeros = [
            np.zeros((NCORES * sh[0],) + sh[1:], dt) for sh, dt in zero_shapes
        ]
        out_arrs = sharded(*concat_in, *concat_zeros)
        return [
            {name: np.asarray(out_arrs[i]).reshape(
                (NCORES,) + zero_shapes[i][0])[c]
             for i, name in enumerate(out_names)}
            for c in range(NCORES)
        ]

    _CACHE["runner"] = run
    return run


def _combine(results):
    out = np.zeros((B, S, HID), np.float32)
    for b in range(B):
        acc = np.zeros((HID, S), np.float32)
        sumsq = np.zeros(S, np.float32)
        for tp in range(4):
            r = results[b * 4 + tp]
            acc += r["outt"]
            # ztout [hf][p][pair][t] -> z rows pair*128+p, cols hf*HALF+t
            z = np.transpose(r["ztout"], (2, 1, 0, 3)).astype(np.float32)
            sumsq += (z * z).sum(axis=(0, 1)).reshape(S)
        scale = 1.0 / np.sqrt(sumsq / INTER + EPS)
        out[b] = (acc * scale[None, :]).T
    return out


def kernel(**inputs):
    if "nc" not in _CACHE:
        _CACHE["nc"] = _build_program()
    nc = _CACHE["nc"]
    in_maps = _host_prep(inputs)
    results = _get_runner(nc)(in_maps)
    return _combine(results)

